# revision 1
# baseline (speedup 1.0000x reference)
"""Trainium2 Bass kernel: ChebWavelet GNN message passing (Chebyshev K=3).

Reference computation:
    T0 = X; T1 = L@X; T2 = 2*L@T1 - T0; out = concat([T0,T1,T2], -1) @ W + b
with L sparse in COO form (edge_row, edge_col, edge_val).

Distribution: nodes are relabeled by a degree-balancing permutation and
sharded row-wise over 8 NeuronCores.  Per spmm pass each core:
  - DMA-gathers single bf16 table rows (128B payload, 256B row stride;
    the table is split lo/hi so indices fit int16) for the edges of its
    128-row destination tiles; edges are streamed per 4-tile supertile
    with one dma_gather per (supertile, table-half);
  - builds a narrow row-major {edge x row-offset} indicator on the
    VectorEngine (one is_equal + one multiply-by-val per supertile):
    edges are pre-sorted by destination row and spread over chunks by
    row-quantile, so each 128-edge chunk spans only a W-row window;
    all operands keep packed bf16 last dims (DVE 2x mode);
  - segment-sums via TensorEngine matmul (messages stationary, indicator
    moving) accumulating into a PE-zeroed PSUM tile at per-chunk static
    row offsets -> f-major [64, 128] tiles.  Value scaling rides the
    matmul contraction.
T1 slices are exchanged with an AllGather collective (the only comms).
The Chebyshev recombination 2*L@T1 - T0 is folded into the linear layer:
    out = X@(W0-W2) + T1@W1 + (L@T1)@(2*W2) + b
so pass 2's raw spmm result feeds the final matmul directly.
"""

import numpy as np
import ml_dtypes

import concourse.bacc as bacc
import concourse.mybir as mybir
from concourse.bass_types import AP


F = 64          # feature dim
FO = 128        # output feature dim
TILE_R = 128    # rows per destination tile
NBUF = 4        # psum-tile pipeline depth
ST = 2          # tiles per supertile (gather batch)


class Cfg:
    def __init__(self, n_nodes, n_edges, n_cores):
        assert n_nodes % (n_cores * TILE_R) == 0
        self.N = n_nodes
        self.E = n_edges
        self.NC = n_cores
        self.RPC = n_nodes // n_cores           # rows per core
        self.TPC = self.RPC // TILE_R           # tiles per core
        self.HALF = n_nodes // 2
        assert self.TPC % (2 * ST) == 0
        assert self.HALF <= 32768               # int16 gather indices


def _chunk_layout(K):
    """Chunk numbering: supertile-major, then half, then tile."""
    TPC = K.shape[0]
    NST = TPC // ST
    gbase = np.zeros(NST + 1, np.int64)
    tbase = np.zeros((TPC, 2), np.int64)
    c = 0
    for sg in range(NST):
        gbase[sg] = c
        for h in range(2):
            for ti in range(ST):
                t = sg * ST + ti
                tbase[t, h] = c
                c += int(K[t, h])
    gbase[NST] = c
    return gbase, tbase


def _raw_gather(gp, out_ap, in_ap, idxs_ap, num_idxs, reg, elem_size):
    """dma_gather with elem_size_bytes not a multiple of 256 (the bass
    wrapper's 256B assert is a transpose-mode restriction; non-transpose
    gathers take any elem size -- verified on HW).  Table rows must still
    be 256B-strided (stride_bytes_256=1)."""
    inst = mybir.InstDMAGatherAnt(
        name=gp.bass.get_next_instruction_name(),
        ins=[*gp.lower_ap_dma(in_ap, for_custom_bir_dma=True),
             gp.lower_ap(idxs_ap),
             gp.lower_val_access(reg)],
        outs=[gp.lower_ap(out_ap)],
        transpose=False, num_idxs=num_idxs, elem_size=elem_size,
        stride_bytes_256=1, gen_mode=0, single_packet=True, queue_num=0,
        sbuf_tokens_per_rank=0, sbuf_free_dim_per_rank=0,
        sbuf_free_dim_pad_per_rank=0, sbuf_byte_offset=0,
    )
    return gp.add_instruction(inst)


def _preprocess(X, edge_row, edge_col, edge_val, cfg):
    """Host-side: permutation, edge bucketing, padded per-core streams."""
    N, NC, RPC, TPC = cfg.N, cfg.NC, cfg.RPC, cfg.TPC
    G = NC * TPC                                 # global tile count
    NST = TPC // ST

    deg = np.bincount(edge_row, minlength=N)
    order = np.argsort(-deg, kind="stable")
    k = np.arange(N)
    rnd, pos = k // G, k % G
    tile_of = np.where(rnd % 2 == 0, pos, G - 1 - pos)
    sigma = np.empty(N, dtype=np.int64)
    sigma[order] = tile_of * TILE_R + rnd        # permuted node index

    r = sigma[edge_row]
    c = sigma[edge_col]
    v = np.asarray(edge_val, dtype=np.float32)

    gtile = r // TILE_R                          # destination tile (global)
    t_in = gtile % TPC                           # tile within core
    half = (c >= cfg.HALF).astype(np.int64)      # gather-table half
    # stream slot: (core, supertile, half, tile-within-supertile)
    slot = ((gtile // TPC) * NST + t_in // ST) * (2 * ST) \
        + half * ST + (t_in % ST)
    key = slot * TILE_R + (r % TILE_R)
    order_e = np.argsort(key, kind="stable")
    slot_s = slot[order_e]

    cnt_th = np.bincount(gtile * 2 + half, minlength=G * 2)
    K = np.maximum((cnt_th.reshape(NC, TPC, 2) + TILE_R - 1) // TILE_R,
                   1).max(axis=0)                # [TPC, 2]
    gbase, tbase = _chunk_layout(K)
    CH = int(gbase[-1])
    L = CH * TILE_R

    # per-slot chunk base (in chunks) and per-slot counts
    t_of_slot = np.empty(NC * NST * 2 * ST, np.int64)
    h_of_slot = np.empty(NC * NST * 2 * ST, np.int64)
    s = np.arange(NC * NST * 2 * ST)
    sl = s % (2 * ST)
    h_of_slot = sl // ST
    t_of_slot = (s // (2 * ST) % NST) * ST + sl % ST
    slot_cbase = tbase[t_of_slot, h_of_slot]     # [n_slots] chunk base
    slot_K = K[t_of_slot, h_of_slot]

    cnt_slot = np.bincount(slot, minlength=NC * NST * 2 * ST)
    starts = np.concatenate([[0], np.cumsum(cnt_slot)[:-1]])
    rank = np.arange(cfg.E) - starts[slot_s]
    cnt_run = cnt_slot[slot_s]
    k_run = slot_K[slot_s]
    chunk_i = (rank * k_run) // np.maximum(cnt_run, 1)
    start_c = (chunk_i * cnt_run + k_run - 1) // k_run
    core_of = slot_s // (NST * 2 * ST)
    pos_local = (slot_cbase[slot_s] * TILE_R
                 + chunk_i * TILE_R + rank - start_c)

    idx_arr = np.zeros((NC, L), np.int16)
    val_arr = np.zeros((NC, L), np.float32)
    rloc_arr = np.zeros((NC, L), np.int32)
    idx_arr[core_of, pos_local] = (
        c[order_e] - half[order_e] * cfg.HALF).astype(np.int16)
    val_arr[core_of, pos_local] = v[order_e]
    rloc_arr[core_of, pos_local] = (r[order_e] % TILE_R).astype(np.int32)

    # Per-chunk row window shared across cores (baked into the program).
    rl3 = rloc_arr.reshape(NC, CH, TILE_R)
    filled = np.zeros((NC, L), bool)
    filled[core_of, pos_local] = True
    f3 = filled.reshape(NC, CH, TILE_R)
    rmin_sh = np.where(f3, rl3, 1 << 20).min(axis=2).min(axis=0)   # [CH]
    rmax_sh = np.where(f3, rl3, -1).max(axis=2).max(axis=0)        # [CH]
    empty = rmax_sh < 0
    rmin_sh = np.where(empty, 0, np.minimum(rmin_sh, 1 << 19))
    rmax_sh = np.maximum(rmax_sh, rmin_sh)
    span = int((rmax_sh - rmin_sh).max()) + 1
    W = max(16, (span + 7) // 8 * 8)
    rbase = np.minimum(rmin_sh, TILE_R - W)           # [CH]
    rl3 = np.where(f3, rl3, rmin_sh[None, :, None])
    rloc_local = rl3 - rbase[None, :, None]
    assert rloc_local.min() >= 0 and rloc_local.max() < W, (
        rloc_local.min(), rloc_local.max(), W)

    # device layouts
    idx_w = idx_arr.reshape(NC, L // 16, 16).transpose(0, 2, 1)
    idx_w = np.tile(idx_w, (1, 8, 1)).copy()
    val_em = val_arr.reshape(NC, CH, TILE_R).transpose(0, 2, 1)
    val_em = np.ascontiguousarray(val_em.astype(ml_dtypes.bfloat16))
    rloc_em = rloc_local.transpose(0, 2, 1)
    rloc_em = np.ascontiguousarray(rloc_em.astype(ml_dtypes.bfloat16))

    Xp = np.empty((N, F), np.float32)
    Xp[sigma] = np.asarray(X, np.float32)
    return dict(sigma=sigma, K=K, CH=CH, L=L, Xp=Xp, W=W, rbase=rbase,
                idx_w=idx_w, val_em=val_em, rloc_em=rloc_em)


def _build(cfg, K, W, rbase):
    """Emit the SPMD Bass program (identical for all cores)."""
    NC, RPC, TPC, HALF = cfg.NC, cfg.RPC, cfg.TPC, cfg.HALF
    NST = TPC // ST
    gbase, tbase = _chunk_layout(K)
    CH = int(gbase[-1])
    L = CH * TILE_R
    GMAX = int(max(gbase[sg + 1] - gbase[sg] for sg in range(NST)))
    f32, bf16, i16 = mybir.dt.float32, mybir.dt.bfloat16, mybir.dt.int16
    # sub-gather counts (gathers are capped at 8 chunks = 1024 descs)
    nsub_sg = []
    for sg in range(NST):
        tot = 0
        for h in range(2):
            nch = int(tbase[sg * ST + ST - 1, h]
                      + K[sg * ST + ST - 1, h] - tbase[sg * ST, h])
            tot += (nch + 7) // 8
        nsub_sg.append(tot)
    subtot = {}
    acc = [0, 0, 0]
    for sgg in range(2 * NST):
        acc[sgg % 3] += nsub_sg[sgg % NST]
        subtot[sgg] = acc[sgg % 3]

    max_ni = max(
        int(tbase[sg * ST + ST - 1, h] + K[sg * ST + ST - 1, h]
            - tbase[sg * ST, h]) * TILE_R
        for sg in range(NST) for h in range(2))
    nc = bacc.Bacc(trn_type="TRN2", num_devices=NC,
                   dynamic_dma_scratch_size=max_ni * 16 + 4096)

    xpad = nc.declare_dram_parameter("xpad", [cfg.N, 2 * F], bf16,
                                     isOutput=False)
    xptb = nc.declare_dram_parameter("xptb", [F, RPC], bf16, isOutput=False)
    idx = nc.declare_dram_parameter("idx", [128, L // 16], i16, isOutput=False)
    val = nc.declare_dram_parameter("val", [128, CH], bf16, isOutput=False)
    rloc = nc.declare_dram_parameter("rloc", [128, CH], bf16, isOutput=False)
    iotar = nc.declare_dram_parameter("iotar", [128, W * GMAX], bf16,
                                      isOutput=False)
    wb = nc.declare_dram_parameter("wb", [F, 3 * FO], bf16, isOutput=False)
    brep = nc.declare_dram_parameter("brep", [128, FO], f32, isOutput=False)
    ident = nc.declare_dram_parameter("ident", [F, F], f32, isOutput=False)
    out = nc.declare_dram_parameter("out", [RPC, FO], f32, isOutput=True)

    t1pad = nc.dram_tensor("t1pad", [RPC, 2 * F], bf16)
    t1fpad = nc.dram_tensor("t1fpad", [cfg.N, 2 * F], bf16,
                            addr_space="Shared")

    from contextlib import ExitStack
    with ExitStack() as ctx:
        def sb(name, shape, dt):
            return ctx.enter_context(nc.sbuf_tensor(name, shape, dt))

        def ps(name, shape):
            return ctx.enter_context(
                nc.psum_tensor(name, shape, mybir.dt.float32))

        idx_sb = sb("idx_sb", [128, L // 16], i16)
        val_sb = sb("val_sb", [128, CH], bf16)
        rloc_sb = sb("rloc_sb", [128, CH], bf16)
        iota_sb = sb("iota_sb", [128, W * GMAX], bf16)
        xptb_sb = sb("xptb_sb", [F, RPC], bf16)
        wb_sb = sb("wb_sb", [F, 3 * FO], bf16)
        brep_sb = sb("brep_sb", [128, FO], f32)
        ident_sb = sb("ident_sb", [F, F], f32)
        t1f_sb = sb("t1f_sb", [F, RPC], f32)
        t1fb_sb = sb("t1fb_sb", [F, RPC], bf16)
        dst_sb = sb("dst_sb", [128, 3 * GMAX * F], bf16)
        ind_sb = sb("ind_sb", [128, 3 * GMAX * W], bf16)
        zero_sb = sb("zero_sb", [128, FO], bf16)
        t1row_sb = sb("t1row_sb", [128, 2 * F], bf16)
        s2fb_sb = sb("s2fb_sb", [F, 2 * FO], bf16)
        out_sb = sb("out_sb", [128, 2 * FO], f32)

        seg_ps = [ps(f"seg_ps{i}", [F, FO]) for i in range(NBUF)]
        tr_ps = [ps(f"tr_ps{i}", [128, F]) for i in range(2)]
        out_ps = [ps(f"out_ps{i}", [128, FO]) for i in range(2)]

        sem = {name: ctx.enter_context(nc.semaphore(name)) for name in
               ["ld", "g0", "g1", "g2", "sc", "seg", "t1e", "t1b",
                "tr", "trev", "t1w0", "t1w1", "cc", "s2", "o",
                "oev", "ow0", "ow1", "zz"]}
        gsems = [sem["g0"], sem["g1"], sem["g2"]]

        def psum_war_wait(eng, ei):
            pei = ei - NBUF
            if pei < 0:
                return
            if pei < TPC:
                eng.wait_ge(sem["t1e"], pei + 1)
            else:
                eng.wait_ge(sem["s2"], pei - TPC + 1)

        def dve_extras(ve, p, t):
            b2 = t % 2
            ve.wait_ge(sem["o"], t + 1)
            if t >= 2:
                ve.wait_ge(sem["ow0" if b2 == 0 else "ow1"],
                           16 * (t // 2))
            ve.tensor_tensor(out_sb[:, b2 * FO:(b2 + 1) * FO],
                             out_ps[b2][:, :], brep_sb[:, :],
                             mybir.AluOpType.add).then_inc(sem["oev"], 1)

        with nc.Block() as block:

            @block.sync
            def _(sync):
                for dst, src in [
                    (idx_sb[:, :], idx[:, :]), (val_sb[:, :], val[:, :]),
                    (rloc_sb[:, :], rloc[:, :]), (iota_sb[:, :], iotar[:, :]),
                    (xptb_sb[:, :], xptb[:, :]), (wb_sb[:, :], wb[:, :]),
                    (brep_sb[:, :], brep[:, :]), (ident_sb[:, :], ident[:, :]),
                ]:
                    sync.dma_start(dst, src).then_inc(sem["ld"], 16)
                # zero-fill t1pad's pad columns once (the collective copies
                # the full rows; the pad bytes are otherwise never written)
                zsrc = zero_sb[:, 0:F]
                zsrc3 = AP(zsrc.tensor, zsrc.offset,
                           [zsrc.ap[0], [0, RPC // 128], [1, F]])
                zdst = t1pad[:, F:2 * F]
                zdst3 = AP(zdst.tensor, zdst.offset,
                           [[2 * F * (RPC // 128), 128],
                            [2 * F, RPC // 128], [1, F]])
                sync.wait_ge(sem["zz"], 1)
                sync.dma_start(zdst3, zsrc3).then_inc(sem["ld"], 16)
                for t in range(TPC):
                    b = t % 2
                    sync.wait_ge(sem["trev"], t + 1)
                    sync.dma_start(
                        t1pad[t * TILE_R:(t + 1) * TILE_R, 0:F],
                        t1row_sb[:, b * F:(b + 1) * F],
                    ).then_inc(sem["t1w0" if b == 0 else "t1w1"], 16)
                for t in range(TPC):
                    b = t % 2
                    sync.wait_ge(sem["oev"], t + 1)
                    sync.dma_start(
                        out[t * TILE_R:(t + 1) * TILE_R, :],
                        out_sb[:, b * FO:(b + 1) * FO],
                    ).then_inc(sem["ow0" if b == 0 else "ow1"], 16)

            @block.gpsimd
            def _(gp):
                MAXCH = 8      # 1024 descriptors per gather call max
                ni_regs = {}
                for sg in range(NST):
                    for h in range(2):
                        nch = int(tbase[sg * ST + ST - 1, h]
                                  + K[sg * ST + ST - 1, h]
                                  - tbase[sg * ST, h])
                        for c0 in range(0, nch, MAXCH):
                            ni = min(MAXCH, nch - c0) * TILE_R
                            if ni not in ni_regs:
                                ni_regs[ni] = gp.to_reg(ni)
                gp.wait_ge(sem["ld"], 16 * 9)
                for p in range(2):
                    if p == 1:
                        gp.wait_ge(sem["t1w0"], 16 * (TPC // 2))
                        gp.wait_ge(sem["t1w1"], 16 * (TPC // 2))
                        gp.collective_compute(
                            "AllGather", mybir.AluOpType.bypass,
                            replica_groups=[list(range(NC))],
                            ins=[t1pad[:, :]], outs=[t1fpad[:, :]],
                        ).then_inc(sem["cc"], 1)
                        gp.wait_ge(sem["cc"], 1)
                    table = xpad if p == 0 else t1fpad
                    for sg in range(NST):
                        sgg = p * NST + sg
                        b = sgg % 3
                        if sgg >= 3:
                            gp.wait_ge(sem["seg"], (sgg - 2) * ST)
                        for h in range(2):
                            cb = int(tbase[sg * ST, h])
                            nch = int(tbase[sg * ST + ST - 1, h]
                                      + K[sg * ST + ST - 1, h] - cb)
                            for c0 in range(0, nch, MAXCH):
                                nsub = min(MAXCH, nch - c0)
                                ni = nsub * TILE_R
                                cks = cb + c0
                                dst = dst_sb[
                                    :, b * GMAX * F + (cks - gbase[sg]) * F:
                                    b * GMAX * F + (cks - gbase[sg] + nsub) * F]
                                dst3 = AP(dst.tensor, dst.offset,
                                          [dst.ap[0], [F, nsub], [1, F]])
                                _raw_gather(
                                    gp, dst3,
                                    table[h * HALF:(h + 1) * HALF, 0:F],
                                    idx_sb[:, cks * 8:cks * 8 + ni // 16],
                                    ni, ni_regs[ni], F,
                                ).then_inc(gsems[b], 16)

            @block.vector
            def _(ve):
                ve.memset(zero_sb[:, :], 0.0)
                ve.drain()
                ve.sem_inc(sem["zz"], 1)
                ve.wait_ge(sem["ld"], 16 * 9)
                for p in range(2):
                    for sg in range(NST):
                        sgg = p * NST + sg
                        b = sgg % 3
                        nch = int(gbase[sg + 1] - gbase[sg])
                        if sgg >= 3:
                            ve.wait_ge(sem["seg"], (sgg - 2) * ST)
                        ind = ind_sb[:, b * GMAX * W:b * GMAX * W + nch * W]
                        ind3 = AP(ind.tensor, ind.offset,
                                  [ind.ap[0], [nch, W], [1, nch]])
                        rl = rloc_sb[:, gbase[sg]:gbase[sg] + nch]
                        rl3 = AP(rl.tensor, rl.offset,
                                 [rl.ap[0], [0, W], [1, nch]])
                        io = iota_sb[:, :]
                        io3 = AP(io.tensor, io.offset,
                                 [io.ap[0], [GMAX, W], [1, nch]])
                        ve.tensor_tensor(ind3, rl3, io3,
                                         mybir.AluOpType.is_equal)
                        ve.drain()
                        vl = val_sb[:, gbase[sg]:gbase[sg] + nch]
                        vl3 = AP(vl.tensor, vl.offset,
                                 [vl.ap[0], [0, W], [1, nch]])
                        ve.tensor_tensor(ind3, ind3, vl3,
                                         mybir.AluOpType.mult
                                         ).then_inc(sem["sc"], 1)
                        if p == 1 and sg >= 1:
                            for t in range((sg - 1) * ST, sg * ST):
                                dve_extras(ve, p, t)
                    if p == 1:
                        for t in range((NST - 1) * ST, TPC):
                            dve_extras(ve, p, t)

            @block.scalar
            def _(sc):
                for p in range(2):
                    for t in range(TPC):
                        ei = p * TPC + t
                        b = ei % NBUF
                        b2 = t % 2
                        sc.wait_ge(sem["seg"], ei + 1)
                        if p == 0:
                            sc.activation(
                                t1f_sb[:, t * TILE_R:(t + 1) * TILE_R],
                                seg_ps[b][:, :],
                                mybir.ActivationFunctionType.Copy,
                            ).then_inc(sem["t1e"], 1)
                            sc.activation(
                                t1fb_sb[:, t * TILE_R:(t + 1) * TILE_R],
                                t1f_sb[:, t * TILE_R:(t + 1) * TILE_R],
                                mybir.ActivationFunctionType.Copy,
                            ).then_inc(sem["t1b"], 1)
                            sc.wait_ge(sem["tr"], t + 1)
                            if t >= 2:
                                sc.wait_ge(sem["t1w0" if b2 == 0 else "t1w1"],
                                           16 * (t // 2))
                            sc.activation(
                                t1row_sb[:, b2 * F:(b2 + 1) * F],
                                tr_ps[b2][:, :],
                                mybir.ActivationFunctionType.Copy,
                            ).then_inc(sem["trev"], 1)
                        else:
                            if t >= 2:
                                sc.wait_ge(sem["o"], t - 1)
                            sc.activation(
                                s2fb_sb[:, b2 * FO:(b2 + 1) * FO],
                                seg_ps[b][:, :],
                                mybir.ActivationFunctionType.Copy,
                            ).then_inc(sem["s2"], 1)

            @block.tensor
            def _(te):
                for p in range(2):
                    for t in range(TPC):
                        ei = p * TPC + t
                        b = ei % NBUF
                        b2 = t % 2
                        sg = t // ST
                        sgg = p * NST + sg
                        te.wait_ge(sem["sc"], sgg + 1)
                        psum_war_wait(te, ei)
                        te.wait_ge(gsems[sgg % 3], 16 * subtot[sgg])
                        te.matmul(seg_ps[b][:, :], zero_sb[:, 0:F],
                                  zero_sb[:, :], start=True, stop=False)
                        gb = int(gbase[sg])
                        nch_sg = int(gbase[sg + 1] - gb)
                        dbase = (sgg % 3) * GMAX * F
                        ibase = (sgg % 3) * GMAX * W
                        for h in range(2):
                            for kk in range(int(K[t, h])):
                                ck = int(tbase[t, h]) + kk
                                rb = int(rbase[ck])
                                off = ck - gb
                                stat = dst_sb[:, dbase + off * F:
                                              dbase + (off + 1) * F]
                                ind = ind_sb[:, ibase + off:ibase + off + 1]
                                ind3 = AP(ind.tensor, ind.offset,
                                          [ind.ap[0], [nch_sg, W]])
                                te.matmul(
                                    seg_ps[b][:, rb:rb + W], stat, ind3,
                                    start=False, stop=False,
                                )
                        te.matmul(seg_ps[b][:, :], zero_sb[:, 0:F],
                                  zero_sb[:, :], start=False, stop=True,
                                  ).then_inc(sem["seg"], 1)
                        if p == 0:
                            te.wait_ge(sem["t1e"], t + 1)
                            if t >= 2:
                                te.wait_ge(sem["trev"], t - 1)
                            te.matmul(
                                tr_ps[b2][:, :],
                                t1f_sb[:, t * TILE_R:(t + 1) * TILE_R],
                                ident_sb[:, :], is_transpose=True,
                            ).then_inc(sem["tr"], 1)
                        else:
                            if t == 0:
                                te.wait_ge(sem["t1b"], TPC)
                                te.wait_ge(sem["ld"], 16 * 9)
                            te.wait_ge(sem["s2"], t + 1)
                            if t >= 2:
                                te.wait_ge(sem["oev"], t - 1)
                            hs = [
                                xptb_sb[:, t * TILE_R:(t + 1) * TILE_R],
                                t1fb_sb[:, t * TILE_R:(t + 1) * TILE_R],
                                s2fb_sb[:, b2 * FO:(b2 + 1) * FO],
                            ]
                            for j in range(3):
                                mm = te.matmul(
                                    out_ps[b2][:, :], hs[j],
                                    wb_sb[:, j * FO:(j + 1) * FO],
                                    start=(j == 0), stop=(j == 2),
                                )
                            mm.then_inc(sem["o"], 1)

    if not nc.is_finalized():
        nc.finalize()
    return nc


_CACHE = {}


def _get_program(cfg, K, W, rbase):
    key = (cfg.N, cfg.E, cfg.NC, K.tobytes(), W, rbase.tobytes())
    if key not in _CACHE:
        _CACHE[key] = _build(cfg, K, W, rbase)
    return _CACHE[key]


def _make_in_maps(prep, W_mat, b, cfg):
    K = prep["K"]
    W = prep["W"]
    gbase, _ = _chunk_layout(K)
    NST = cfg.TPC // ST
    GMAX = int(max(gbase[sg + 1] - gbase[sg] for sg in range(NST)))
    W_mat = np.asarray(W_mat, np.float32)
    b = np.asarray(b, np.float32)
    W0, W1, W2 = W_mat[:F], W_mat[F:2 * F], W_mat[2 * F:]
    wb = np.concatenate([W0 - W2, W1, 2.0 * W2],
                        axis=1).astype(ml_dtypes.bfloat16)
    brep = np.broadcast_to(b, (128, FO)).copy().astype(np.float32)
    iotar = np.broadcast_to(
        np.repeat(np.arange(W, dtype=np.float32), GMAX),
        (128, W * GMAX)).astype(ml_dtypes.bfloat16).copy()
    ident = np.eye(F, dtype=np.float32)
    Xp = prep["Xp"]
    xpad = np.zeros((cfg.N, 2 * F), ml_dtypes.bfloat16)
    xpad[:, 0:F] = Xp.astype(ml_dtypes.bfloat16)
    in_maps = []
    for c in range(cfg.NC):
        xptb = np.ascontiguousarray(
            Xp[c * cfg.RPC:(c + 1) * cfg.RPC].T.astype(ml_dtypes.bfloat16))
        in_maps.append(dict(
            xpad=xpad, xptb=xptb,
            idx=prep["idx_w"][c], val=prep["val_em"][c],
            rloc=prep["rloc_em"][c],
            iotar=iotar, wb=wb, brep=brep, ident=ident,
        ))
    return in_maps


def kernel(X, edge_row, edge_col, edge_val, W, b):
    X = np.asarray(X, np.float32)
    edge_row = np.asarray(edge_row, np.int32)
    edge_col = np.asarray(edge_col, np.int32)
    edge_val = np.asarray(edge_val, np.float32)
    cfg = Cfg(X.shape[0], edge_row.shape[0], 8)
    prep = _preprocess(X, edge_row, edge_col, edge_val, cfg)
    nc = _get_program(cfg, prep["K"], prep["W"], prep["rbase"])
    in_maps = _make_in_maps(prep, W, b, cfg)

    from concourse.bass_utils import run_bass_kernel_spmd
    res = run_bass_kernel_spmd(nc, in_maps, list(range(cfg.NC)))
    out_perm = np.concatenate([res.results[i]["out"] for i in range(cfg.NC)],
                              axis=0)
    return np.ascontiguousarray(out_perm[prep["sigma"]]).astype(np.float32)

